# revision 5
# baseline (speedup 1.0000x reference)
"""Trainium2 Bass kernel for CALayer with top-k channel masking.

Computation (per batch item):
  y = mean(x, spatial)                    # [C]
  h = relu(w1 @ y + b1)                   # [C/R]
  a = sigmoid(w2 @ h + b2)                # [C]
  idx = sort(top_k(a, 128).indices)       # ascending channel ids
  out = a[idx, None, None] * x[idx]       # [128, H, W]

Strategy: data-parallel over batch (32 items -> 8 cores x 4). Everything
on-device per core:
  - x[b] loaded once to SBUF [128 part, 2 chunk, 4096 spatial]; means via one
    DVE reduce (1/HW folded into prepacked w1T).
  - MLP with tiny PE matmuls; ranking done on pre-sigmoid logits z (monotone
    => same selection as sigmoid, better numerics).
  - rank[c] = #{c': z[c'] > z[c]} via PE transpose-broadcast of z +
    DVE tensor_scalar(is_gt) with accum_out.
  - mask m = rank < K; output slot p = exclusive-prefix-sum(m) via matmul
    with strict-upper-triangular constant; unselected slots -> 512 (OOB).
  - xs = x * sigmoid(z) per-partition on ACT.
  - one indirect SBUF->DRAM scatter per chunk with bounds_check=K-1,
    oob_is_err=False: unselected channels are silently dropped, so HBM sees
    only the 128 selected rows.
"""

import os
from contextlib import ExitStack

import numpy as np

import concourse.bass as bass
import concourse.tile as tile
from concourse import bacc, mybir
from concourse.bass_utils import run_bass_kernel_spmd
from concourse.masks import make_identity

N_CORES = 8
B_FULL, C, H, W = 32, 256, 64, 64
NB = B_FULL // N_CORES  # batch items per core
HW = H * W
K = 128  # top-k
P = 128  # partitions
NCH = C // P  # channel chunks
R = 16  # reduction dim
OOB = 512.0  # out-of-bounds slot for unselected channels
F32 = mybir.dt.float32


def _body(ctx: ExitStack, tc: "tile.TileContext", x_d, out_d, w1t_d, w2t_d, b1_d, b2_d, sut_d, ones_d):
    nc = tc.nc
    AF = mybir.ActivationFunctionType
    ALU = mybir.AluOpType

    cpool = ctx.enter_context(tc.tile_pool(name="const", bufs=1))
    xp = ctx.enter_context(tc.tile_pool(name="x", bufs=2))
    xsp = ctx.enter_context(tc.tile_pool(name="xs", bufs=2))
    sp = ctx.enter_context(tc.tile_pool(name="small", bufs=2))
    gp = ctx.enter_context(tc.tile_pool(name="g", bufs=2))
    pp = ctx.enter_context(tc.tile_pool(name="ps", bufs=2, space="PSUM"))
    zp = ctx.enter_context(tc.tile_pool(name="zrep", bufs=2, space="PSUM"))

    # constants / weights (replicated on every core)
    w1t_sb = cpool.tile([P, NCH, R], F32)
    nc.sync.dma_start(w1t_sb[:], w1t_d.ap().rearrange("(k p) r -> p k r", p=P))
    w2t_sb = cpool.tile([R, C], F32)
    nc.sync.dma_start(w2t_sb[:], w2t_d.ap())
    b1_sb = cpool.tile([R, 1], F32)
    nc.sync.dma_start(b1_sb[:], b1_d.ap())
    b2_sb = cpool.tile([P, NCH], F32)
    nc.sync.dma_start(b2_sb[:], b2_d.ap().rearrange("k p -> p k"))
    sut_sb = cpool.tile([P, P], F32)
    nc.sync.dma_start(sut_sb[:], sut_d.ap())
    ones_sb = cpool.tile([P, P], F32)
    nc.sync.dma_start(ones_sb[:], ones_d.ap())
    ident_sb = cpool.tile([P, P], F32)
    make_identity(nc, ident_sb[:])

    out_flat = out_d.ap().rearrange("b k f -> (b k) f")  # [NB*K, HW], offset 0

    for b in range(NB):
        # load x[b]: channel c = k*P + p  ->  xt[p, k, :]
        xt = xp.tile([P, NCH, HW], F32, tag="x")
        nc.sync.dma_start(xt[:], x_d.ap()[b].rearrange("(k p) f -> p k f", p=P))

        # spatial sums (mean folded into w1t prescale)
        y2 = sp.tile([P, NCH], F32, tag="y")
        nc.vector.reduce_sum(out=y2[:], in_=xt[:], axis=mybir.AxisListType.X)

        # h = relu(w1 @ y + b1)
        ht_ps = pp.tile([R, 1], F32, tag="ht")
        nc.tensor.matmul(ht_ps[:], lhsT=w1t_sb[:, 0, :], rhs=y2[:, 0:1], start=True, stop=False)
        nc.tensor.matmul(ht_ps[:], lhsT=w1t_sb[:, 1, :], rhs=y2[:, 1:2], start=False, stop=True)
        ht_sb = sp.tile([R, 1], F32, tag="htsb")
        nc.scalar.activation(ht_sb[:], ht_ps[:], AF.Relu, bias=b1_sb[:])

        # z = w2 @ h  (logits; bias added in sigmoid step / compare uses z+b2 via zb)
        z_ps = pp.tile([P, NCH], F32, tag="z")
        for k in range(NCH):
            nc.tensor.matmul(z_ps[:, k : k + 1], lhsT=w2t_sb[:, k * P : (k + 1) * P], rhs=ht_sb[:], start=True, stop=True)
        # zb = z + b2 (full logit, used for ranking), a = sigmoid(zb) (scaling)
        zb_sb = sp.tile([P, NCH], F32, tag="zb")
        nc.vector.tensor_tensor(out=zb_sb[:], in0=z_ps[:], in1=b2_sb[:], op=ALU.add)
        a_sb = sp.tile([P, NCH], F32, tag="a")
        for k in range(NCH):
            nc.scalar.activation(a_sb[:, k : k + 1], z_ps[:, k : k + 1], AF.Sigmoid, bias=b2_sb[:, k : k + 1])

        # replicate zb across partitions: zrep[p, c'] = zb[c']
        zrep_ps = zp.tile([P, C], F32, tag="zrep")
        for k in range(NCH):
            nc.tensor.transpose(zrep_ps[:, k * P : (k + 1) * P], in_=zb_sb[:, k : k + 1].to_broadcast([P, P]), identity=ident_sb[:])

        # rank[c] = #{c': zb[c'] > zb[c]}  (compare + count fused via accum_out)
        rank = sp.tile([P, NCH], F32, tag="rank")
        for k in range(NCH):
            g = gp.tile([P, C], F32, tag="g")
            nc.vector.tensor_scalar(g[:], zrep_ps[:], zb_sb[:, k : k + 1], None, ALU.is_gt, ALU.add, accum_out=rank[:, k : k + 1])

        # mask: selected iff rank < K
        m = sp.tile([P, NCH], F32, tag="m")
        nc.vector.tensor_scalar(m[:], rank[:], float(K) - 0.5, None, ALU.is_lt)

        # output slot p[c] = #{selected c' < c} (exclusive prefix over channels)
        p_ps = pp.tile([P, NCH], F32, tag="p")
        nc.tensor.matmul(p_ps[:, 0:1], lhsT=sut_sb[:], rhs=m[:, 0:1], start=True, stop=True)
        nc.tensor.matmul(p_ps[:, 1:2], lhsT=ones_sb[:], rhs=m[:, 0:1], start=True, stop=False)
        nc.tensor.matmul(p_ps[:, 1:2], lhsT=sut_sb[:], rhs=m[:, 1:2], start=False, stop=True)

        # q = p + OOB*(1-m); cast to int32 for the scatter offset table
        tmp = sp.tile([P, NCH], F32, tag="tmp")
        nc.vector.tensor_scalar(tmp[:], m[:], -OOB, OOB, ALU.mult, ALU.add)
        qf = sp.tile([P, NCH], F32, tag="qf")
        nc.vector.tensor_tensor(out=qf[:], in0=p_ps[:], in1=tmp[:], op=ALU.add)
        qi = sp.tile([P, NCH], mybir.dt.int32, tag="qi")
        nc.vector.tensor_copy(qi[:], qf[:])

        # scale all channels by attn weight (per-partition scale on ACT)
        xs = xsp.tile([P, NCH, HW], F32, tag="xs")
        for k in range(NCH):
            nc.scalar.activation(xs[:, k, :], xt[:, k, :], AF.Copy, scale=a_sb[:, k : k + 1])

        # scatter selected channels to out[b, slot, :]; OOB slots dropped
        for k in range(NCH):
            nc.gpsimd.indirect_dma_start(
                out=out_flat,
                out_offset=bass.IndirectOffsetOnAxis(ap=qi[:, k : k + 1], axis=0),
                in_=xs[:, k, :],
                in_offset=None,
                element_offset=b * K * HW,
                bounds_check=K - 1,
                oob_is_err=False,
            )


def build_nc():
    nc = bacc.Bacc("TRN2", target_bir_lowering=False, debug=False, num_devices=N_CORES)
    x_d = nc.dram_tensor("x", [NB, C, HW], F32, kind="ExternalInput")
    w1t_d = nc.dram_tensor("w1t", [C, R], F32, kind="ExternalInput")
    w2t_d = nc.dram_tensor("w2t", [R, C], F32, kind="ExternalInput")
    b1_d = nc.dram_tensor("b1", [R, 1], F32, kind="ExternalInput")
    b2_d = nc.dram_tensor("b2", [NCH, P], F32, kind="ExternalInput")
    sut_d = nc.dram_tensor("sut", [P, P], F32, kind="ExternalInput")
    ones_d = nc.dram_tensor("ones", [P, P], F32, kind="ExternalInput")
    out_d = nc.dram_tensor("out", [NB, K, HW], F32, kind="ExternalOutput")
    with tile.TileContext(nc) as tc:
        with ExitStack() as ctx:
            _body(ctx, tc, x_d, out_d, w1t_d, w2t_d, b1_d, b2_d, sut_d, ones_d)
    nc.compile()
    return nc


def make_in_maps(x, w1, b1, w2, b2):
    """Per-core input dicts. x: [32, 256, 64, 64] f32."""
    w1t = np.ascontiguousarray(w1.T).astype(np.float32) / float(HW)  # [C, R], mean folded in
    w2t = np.ascontiguousarray(w2.T).astype(np.float32)  # [R, C]
    b1c = b1.astype(np.float32).reshape(R, 1)
    b2c = b2.astype(np.float32).reshape(NCH, P)
    sut = np.triu(np.ones((P, P), np.float32), k=1)  # sut[k, m] = 1 iff k < m
    ones = np.ones((P, P), np.float32)
    xr = np.ascontiguousarray(x.astype(np.float32).reshape(B_FULL, C, HW))
    in_maps = []
    for i in range(N_CORES):
        in_maps.append(
            {
                "x": np.ascontiguousarray(xr[i * NB : (i + 1) * NB]),
                "w1t": w1t,
                "w2t": w2t,
                "b1": b1c,
                "b2": b2c,
                "sut": sut,
                "ones": ones,
            }
        )
    return in_maps


def _install_ntff_hook():
    """Bridge the missing antenv.axon_hooks module so run_bass_kernel_spmd
    trace=True can capture NTFF profiles via the axon PJRT .so."""
    import sys
    import types

    if "antenv.axon_hooks" in sys.modules:
        return
    try:
        if "/root/.axon_site" not in sys.path:
            sys.path.insert(0, "/root/.axon_site")
        from trn_agent_boot.trn_boot import _ntff_profile_via_ctypes

        hook = _ntff_profile_via_ctypes("/opt/axon/libaxon_pjrt.so")
        mod = types.ModuleType("antenv.axon_hooks")
        mod.get_axon_ntff_profile_hook = lambda: hook
        mod.set_axon_ntff_profile_hook = lambda h: None
        sys.modules["antenv.axon_hooks"] = mod
    except Exception as e:  # degrade to no tracing
        print("ntff hook install failed:", e)


_NC_CACHE = {}


def get_nc():
    if "nc" not in _NC_CACHE:
        _NC_CACHE["nc"] = build_nc()
    return _NC_CACHE["nc"]


def kernel(x, w1, b1, w2, b2, topk, _trace=False, **_ignored):
    assert int(topk) == K, f"kernel hardcodes topk={K}, got {topk}"
    assert x.shape == (B_FULL, C, H, W)
    nc = get_nc()
    if _trace:
        _install_ntff_hook()
    in_maps = make_in_maps(np.asarray(x), np.asarray(w1), np.asarray(b1), np.asarray(w2), np.asarray(b2))
    res = run_bass_kernel_spmd(nc, in_maps, core_ids=list(range(N_CORES)), trace=_trace)
    outs = [res.results[i]["out"].reshape(NB, K, H, W) for i in range(N_CORES)]
    full = np.concatenate(outs, axis=0).astype(np.float32)
    if _trace:
        return full, res
    return full


# revision 7
# speedup vs baseline: 1.0227x; 1.0227x over previous
"""Trainium2 Bass kernel for CALayer with top-k channel masking.

Computation (per batch item):
  y = mean(x, spatial)                    # [C]
  h = relu(w1 @ y + b1)                   # [C/R]
  a = sigmoid(w2 @ h + b2)                # [C]
  idx = sort(top_k(a, 128).indices)       # ascending channel ids
  out = a[idx, None, None] * x[idx]       # [128, H, W]

Strategy: data-parallel over batch (32 items -> 8 cores x 4). Everything
on-device per core:
  - x[b] loaded once to SBUF [128 part, 2 chunk, 4096 spatial]; means via one
    DVE reduce (1/HW folded into prepacked w1T).
  - MLP with tiny PE matmuls; ranking done on pre-sigmoid logits z (monotone
    => same selection as sigmoid, better numerics).
  - rank[c] = #{c': z[c'] > z[c]} via PE transpose-broadcast of z +
    DVE tensor_scalar(is_gt) with accum_out.
  - mask m = rank < K; output slot p = exclusive-prefix-sum(m) via matmul
    with strict-upper-triangular constant; unselected slots -> 512 (OOB).
  - xs = x * sigmoid(z) per-partition on ACT.
  - one indirect SBUF->DRAM scatter per chunk with bounds_check=K-1,
    oob_is_err=False: unselected channels are silently dropped, so HBM sees
    only the 128 selected rows.
"""

import os
from contextlib import ExitStack

import numpy as np

import concourse.bass as bass
import concourse.tile as tile
from concourse import bacc, mybir
from concourse.bass_utils import run_bass_kernel_spmd
from concourse.masks import make_identity

N_CORES = 8
B_FULL, C, H, W = 32, 256, 64, 64
NB = B_FULL // N_CORES  # batch items per core
HW = H * W
K = 128  # top-k
P = 128  # partitions
NCH = C // P  # channel chunks
R = 16  # reduction dim
OOB = 512.0  # out-of-bounds slot for unselected channels
F32 = mybir.dt.float32


def _body(ctx: ExitStack, tc: "tile.TileContext", x_d, out_d, w1t_d, w2t_d, b1_d, b2_d, sut_d, ones_d):
    nc = tc.nc
    AF = mybir.ActivationFunctionType
    ALU = mybir.AluOpType

    cpool = ctx.enter_context(tc.tile_pool(name="const", bufs=1))
    xp = ctx.enter_context(tc.tile_pool(name="x", bufs=3))
    xsp = ctx.enter_context(tc.tile_pool(name="xs", bufs=2))
    sp = ctx.enter_context(tc.tile_pool(name="small", bufs=2))
    gp = ctx.enter_context(tc.tile_pool(name="g", bufs=2))
    pp = ctx.enter_context(tc.tile_pool(name="ps", bufs=2, space="PSUM"))
    zp = ctx.enter_context(tc.tile_pool(name="zrep", bufs=2, space="PSUM"))

    # constants / weights (replicated on every core); loaded on the ACT HWDGE
    # queue so they don't sit ahead of the big x loads on the sync FIFO
    w1t_sb = cpool.tile([P, NCH, R], F32)
    nc.scalar.dma_start(w1t_sb[:], w1t_d.ap().rearrange("(k p) r -> p k r", p=P))
    w2t_sb = cpool.tile([R, C], F32)
    nc.scalar.dma_start(w2t_sb[:], w2t_d.ap())
    b1_sb = cpool.tile([R, 1], F32)
    nc.scalar.dma_start(b1_sb[:], b1_d.ap())
    b2_sb = cpool.tile([P, NCH], F32)
    nc.scalar.dma_start(b2_sb[:], b2_d.ap().rearrange("k p -> p k"))
    sut_sb = cpool.tile([P, P], F32)
    nc.scalar.dma_start(sut_sb[:], sut_d.ap())
    ones_sb = cpool.tile([P, P], F32)
    nc.scalar.dma_start(ones_sb[:], ones_d.ap())
    ident_sb = cpool.tile([P, P], F32)
    make_identity(nc, ident_sb[:])

    out_flat = out_d.ap().rearrange("b k f -> (b k) f")  # [NB*K, HW], offset 0

    for b in range(NB):
        # load x[b] per chunk: channel c = k*P + p  ->  xt[p, k, :]
        xt = xp.tile([P, NCH, HW], F32, tag="x")
        xs = xsp.tile([P, NCH, HW], F32, tag="xs")
        x_src = x_d.ap()[b].rearrange("(k p) f -> p k f", p=P)
        # spatial sums on ACT (mean folded into w1t prescale); the Copy
        # output is a throwaway write into xs (later overwritten by scale)
        y2 = sp.tile([P, NCH], F32, tag="y")
        for k in range(NCH):
            nc.sync.dma_start(xt[:, k, :], x_src[:, k, :])
            nc.scalar.activation(xs[:, k, :], xt[:, k, :], mybir.ActivationFunctionType.Copy, accum_out=y2[:, k : k + 1])

        # h = relu(w1 @ y + b1)
        ht_ps = pp.tile([R, 1], F32, tag="ht")
        nc.tensor.matmul(ht_ps[:], lhsT=w1t_sb[:, 0, :], rhs=y2[:, 0:1], start=True, stop=False)
        nc.tensor.matmul(ht_ps[:], lhsT=w1t_sb[:, 1, :], rhs=y2[:, 1:2], start=False, stop=True)
        ht_sb = sp.tile([R, 1], F32, tag="htsb")
        nc.scalar.activation(ht_sb[:], ht_ps[:], AF.Relu, bias=b1_sb[:])

        # z = w2 @ h  (logits; bias added in sigmoid step / compare uses z+b2 via zb)
        z_ps = pp.tile([P, NCH], F32, tag="z")
        for k in range(NCH):
            nc.tensor.matmul(z_ps[:, k : k + 1], lhsT=w2t_sb[:, k * P : (k + 1) * P], rhs=ht_sb[:], start=True, stop=True)
        # zb = z + b2 (full logit, used for ranking), a = sigmoid(zb) (scaling)
        zb_sb = sp.tile([P, NCH], F32, tag="zb")
        nc.vector.tensor_tensor(out=zb_sb[:], in0=z_ps[:], in1=b2_sb[:], op=ALU.add)
        a_sb = sp.tile([P, NCH], F32, tag="a")
        for k in range(NCH):
            nc.scalar.activation(a_sb[:, k : k + 1], z_ps[:, k : k + 1], AF.Sigmoid, bias=b2_sb[:, k : k + 1])

        # replicate zb across partitions: zrep[p, c'] = zb[c']
        zrep_ps = zp.tile([P, C], F32, tag="zrep")
        for k in range(NCH):
            nc.tensor.transpose(zrep_ps[:, k * P : (k + 1) * P], in_=zb_sb[:, k : k + 1].to_broadcast([P, P]), identity=ident_sb[:])

        # rank[c] = #{c': zb[c'] > zb[c]}  (compare + count fused via accum_out)
        rank = sp.tile([P, NCH], F32, tag="rank")
        for k in range(NCH):
            g = gp.tile([P, C], F32, tag="g")
            nc.vector.tensor_scalar(g[:], zrep_ps[:], zb_sb[:, k : k + 1], None, ALU.is_gt, ALU.add, accum_out=rank[:, k : k + 1])

        # mask: selected iff rank < K
        m = sp.tile([P, NCH], F32, tag="m")
        nc.vector.tensor_scalar(m[:], rank[:], float(K) - 0.5, None, ALU.is_lt)

        # output slot p[c] = #{selected c' < c} (exclusive prefix over channels)
        p_ps = pp.tile([P, NCH], F32, tag="p")
        nc.tensor.matmul(p_ps[:, 0:1], lhsT=sut_sb[:], rhs=m[:, 0:1], start=True, stop=True)
        nc.tensor.matmul(p_ps[:, 1:2], lhsT=ones_sb[:], rhs=m[:, 0:1], start=True, stop=False)
        nc.tensor.matmul(p_ps[:, 1:2], lhsT=sut_sb[:], rhs=m[:, 1:2], start=False, stop=True)

        # q = p + OOB*(1-m); cast to int32 for the scatter offset table
        tmp = sp.tile([P, NCH], F32, tag="tmp")
        nc.vector.tensor_scalar(tmp[:], m[:], -OOB, OOB, ALU.mult, ALU.add)
        qf = sp.tile([P, NCH], F32, tag="qf")
        nc.vector.tensor_tensor(out=qf[:], in0=p_ps[:], in1=tmp[:], op=ALU.add)
        qi = sp.tile([P, NCH], mybir.dt.int32, tag="qi")
        nc.vector.tensor_copy(qi[:], qf[:])

        # scale all channels by attn weight (per-partition scalar on DVE, 2x mode)
        for k in range(NCH):
            nc.vector.tensor_scalar(xs[:, k, :], xt[:, k, :], a_sb[:, k : k + 1], None, ALU.mult)

        # scatter selected channels to out[b, slot, :]; OOB slots dropped
        for k in range(NCH):
            nc.gpsimd.indirect_dma_start(
                out=out_flat,
                out_offset=bass.IndirectOffsetOnAxis(ap=qi[:, k : k + 1], axis=0),
                in_=xs[:, k, :],
                in_offset=None,
                element_offset=b * K * HW,
                bounds_check=K - 1,
                oob_is_err=False,
            )


def build_nc():
    nc = bacc.Bacc("TRN2", target_bir_lowering=False, debug=False, num_devices=N_CORES)
    x_d = nc.dram_tensor("x", [NB, C, HW], F32, kind="ExternalInput")
    w1t_d = nc.dram_tensor("w1t", [C, R], F32, kind="ExternalInput")
    w2t_d = nc.dram_tensor("w2t", [R, C], F32, kind="ExternalInput")
    b1_d = nc.dram_tensor("b1", [R, 1], F32, kind="ExternalInput")
    b2_d = nc.dram_tensor("b2", [NCH, P], F32, kind="ExternalInput")
    sut_d = nc.dram_tensor("sut", [P, P], F32, kind="ExternalInput")
    ones_d = nc.dram_tensor("ones", [P, P], F32, kind="ExternalInput")
    out_d = nc.dram_tensor("out", [NB, K, HW], F32, kind="ExternalOutput")
    with tile.TileContext(nc) as tc:
        with ExitStack() as ctx:
            _body(ctx, tc, x_d, out_d, w1t_d, w2t_d, b1_d, b2_d, sut_d, ones_d)
    nc.compile()
    return nc


def make_in_maps(x, w1, b1, w2, b2):
    """Per-core input dicts. x: [32, 256, 64, 64] f32."""
    w1t = np.ascontiguousarray(w1.T).astype(np.float32) / float(HW)  # [C, R], mean folded in
    w2t = np.ascontiguousarray(w2.T).astype(np.float32)  # [R, C]
    b1c = b1.astype(np.float32).reshape(R, 1)
    b2c = b2.astype(np.float32).reshape(NCH, P)
    sut = np.triu(np.ones((P, P), np.float32), k=1)  # sut[k, m] = 1 iff k < m
    ones = np.ones((P, P), np.float32)
    xr = np.ascontiguousarray(x.astype(np.float32).reshape(B_FULL, C, HW))
    in_maps = []
    for i in range(N_CORES):
        in_maps.append(
            {
                "x": np.ascontiguousarray(xr[i * NB : (i + 1) * NB]),
                "w1t": w1t,
                "w2t": w2t,
                "b1": b1c,
                "b2": b2c,
                "sut": sut,
                "ones": ones,
            }
        )
    return in_maps


def _install_ntff_hook():
    """Bridge the missing antenv.axon_hooks module so run_bass_kernel_spmd
    trace=True can capture NTFF profiles via the axon PJRT .so."""
    import sys
    import types

    if "antenv.axon_hooks" in sys.modules:
        return
    try:
        if "/root/.axon_site" not in sys.path:
            sys.path.insert(0, "/root/.axon_site")
        from trn_agent_boot.trn_boot import _ntff_profile_via_ctypes

        hook = _ntff_profile_via_ctypes("/opt/axon/libaxon_pjrt.so")
        mod = types.ModuleType("antenv.axon_hooks")
        mod.get_axon_ntff_profile_hook = lambda: hook
        mod.set_axon_ntff_profile_hook = lambda h: None
        sys.modules["antenv.axon_hooks"] = mod
    except Exception as e:  # degrade to no tracing
        print("ntff hook install failed:", e)


_NC_CACHE = {}


def get_nc():
    if "nc" not in _NC_CACHE:
        _NC_CACHE["nc"] = build_nc()
    return _NC_CACHE["nc"]


def kernel(x, w1, b1, w2, b2, topk, _trace=False, **_ignored):
    assert int(topk) == K, f"kernel hardcodes topk={K}, got {topk}"
    assert x.shape == (B_FULL, C, H, W)
    nc = get_nc()
    if _trace:
        _install_ntff_hook()
    in_maps = make_in_maps(np.asarray(x), np.asarray(w1), np.asarray(b1), np.asarray(w2), np.asarray(b2))
    res = run_bass_kernel_spmd(nc, in_maps, core_ids=list(range(N_CORES)), trace=_trace)
    outs = [res.results[i]["out"].reshape(NB, K, H, W) for i in range(N_CORES)]
    full = np.concatenate(outs, axis=0).astype(np.float32)
    if _trace:
        return full, res
    return full


# revision 9
# speedup vs baseline: 1.2792x; 1.2508x over previous
"""Trainium2 Bass kernel for CALayer with top-k channel masking.

Computation (per batch item):
  y = mean(x, spatial)                    # [C]
  h = relu(w1 @ y + b1)                   # [C/R]
  a = sigmoid(w2 @ h + b2)                # [C]
  idx = sort(top_k(a, 128).indices)       # ascending channel ids
  out = a[idx, None, None] * x[idx]       # [128, H, W]

Strategy: data-parallel over batch (32 items -> 8 cores x 4). Everything
on-device per core:
  - x[b] loaded once to SBUF [128 part, 2 chunk, 4096 spatial]; means via one
    DVE reduce (1/HW folded into prepacked w1T).
  - MLP with tiny PE matmuls; ranking done on pre-sigmoid logits z (monotone
    => same selection as sigmoid, better numerics).
  - rank[c] = #{c': z[c'] > z[c]} via PE transpose-broadcast of z +
    DVE tensor_scalar(is_gt) with accum_out.
  - mask m = rank < K; output slot p = exclusive-prefix-sum(m) via matmul
    with strict-upper-triangular constant; unselected slots -> 512 (OOB).
  - xs = x * sigmoid(z) per-partition on ACT.
  - one indirect SBUF->DRAM scatter per chunk with bounds_check=K-1,
    oob_is_err=False: unselected channels are silently dropped, so HBM sees
    only the 128 selected rows.
"""

import os
from contextlib import ExitStack

import numpy as np

import concourse.bass as bass
import concourse.tile as tile
from concourse import bacc, mybir
from concourse.bass_utils import run_bass_kernel_spmd
from concourse.masks import make_identity

N_CORES = 8
B_FULL, C, H, W = 32, 256, 64, 64
NB = B_FULL // N_CORES  # batch items per core
HW = H * W
K = 128  # top-k
P = 128  # partitions
NCH = C // P  # channel chunks
R = 16  # reduction dim
OOB = 512.0  # out-of-bounds slot for unselected channels
F32 = mybir.dt.float32


def _body(ctx: ExitStack, tc: "tile.TileContext", x_d, outs_d, w1t_d, w2t_d, b1_d, b2_d, sut_d, ones_d):
    nc = tc.nc
    AF = mybir.ActivationFunctionType
    ALU = mybir.AluOpType

    cpool = ctx.enter_context(tc.tile_pool(name="const", bufs=1))
    xp = ctx.enter_context(tc.tile_pool(name="x", bufs=3))
    xsp = ctx.enter_context(tc.tile_pool(name="xs", bufs=2))
    sp = ctx.enter_context(tc.tile_pool(name="small", bufs=2))
    gp = ctx.enter_context(tc.tile_pool(name="g", bufs=2))
    pp = ctx.enter_context(tc.tile_pool(name="ps", bufs=2, space="PSUM"))
    zp = ctx.enter_context(tc.tile_pool(name="zrep", bufs=2, space="PSUM"))

    # constants / weights (replicated on every core); loaded on the ACT HWDGE
    # queue so they don't sit ahead of the big x loads on the sync FIFO
    w1t_sb = cpool.tile([P, NCH, R], F32)
    nc.scalar.dma_start(w1t_sb[:], w1t_d.ap().rearrange("(k p) r -> p k r", p=P))
    w2t_sb = cpool.tile([R, C], F32)
    nc.scalar.dma_start(w2t_sb[:], w2t_d.ap())
    b1_sb = cpool.tile([R, 1], F32)
    nc.scalar.dma_start(b1_sb[:], b1_d.ap())
    b2_sb = cpool.tile([P, NCH], F32)
    nc.scalar.dma_start(b2_sb[:], b2_d.ap().rearrange("k p -> p k"))
    sut_sb = cpool.tile([P, P], F32)
    nc.scalar.dma_start(sut_sb[:], sut_d.ap())
    ones_sb = cpool.tile([P, P], F32)
    nc.scalar.dma_start(ones_sb[:], ones_d.ap())
    ident_sb = cpool.tile([P, P], F32)
    make_identity(nc, ident_sb[:])

    trash = cpool.tile([P, HW], F32)  # throwaway write target for means-accum

    for b in range(NB):
        # load x[b] per chunk: channel c = k*P + p  ->  xt[p, k, :]
        xt = xp.tile([P, NCH, HW], F32, tag="x")
        xs = xsp.tile([P, NCH, HW], F32, tag="xs")
        x_src = x_d.ap()[b].rearrange("(k p) f -> p k f", p=P)
        # spatial sums on ACT (mean folded into w1t prescale); the Copy
        # output is a throwaway write into xs (later overwritten by scale)
        y2 = sp.tile([P, NCH], F32, tag="y")
        for k in range(NCH):
            nc.sync.dma_start(xt[:, k, :], x_src[:, k, :])
            nc.scalar.activation(trash[:], xt[:, k, :], mybir.ActivationFunctionType.Copy, accum_out=y2[:, k : k + 1])

        # h = relu(w1 @ y + b1)
        ht_ps = pp.tile([R, 1], F32, tag="ht")
        nc.tensor.matmul(ht_ps[:], lhsT=w1t_sb[:, 0, :], rhs=y2[:, 0:1], start=True, stop=False)
        nc.tensor.matmul(ht_ps[:], lhsT=w1t_sb[:, 1, :], rhs=y2[:, 1:2], start=False, stop=True)
        ht_sb = sp.tile([R, 1], F32, tag="htsb")
        nc.scalar.activation(ht_sb[:], ht_ps[:], AF.Relu, bias=b1_sb[:])

        # z = w2 @ h  (logits; bias added in sigmoid step / compare uses z+b2 via zb)
        z_ps = pp.tile([P, NCH], F32, tag="z")
        for k in range(NCH):
            nc.tensor.matmul(z_ps[:, k : k + 1], lhsT=w2t_sb[:, k * P : (k + 1) * P], rhs=ht_sb[:], start=True, stop=True)
        # zb = z + b2 (full logit, used for ranking), a = sigmoid(zb) (scaling)
        zb_sb = sp.tile([P, NCH], F32, tag="zb")
        nc.vector.tensor_tensor(out=zb_sb[:], in0=z_ps[:], in1=b2_sb[:], op=ALU.add)
        a_sb = sp.tile([P, NCH], F32, tag="a")
        for k in range(NCH):
            nc.scalar.activation(a_sb[:, k : k + 1], z_ps[:, k : k + 1], AF.Sigmoid, bias=b2_sb[:, k : k + 1])

        # replicate zb across partitions: zrep[p, c'] = zb[c']
        zrep_ps = zp.tile([P, C], F32, tag="zrep")
        for k in range(NCH):
            nc.tensor.transpose(zrep_ps[:, k * P : (k + 1) * P], in_=zb_sb[:, k : k + 1].to_broadcast([P, P]), identity=ident_sb[:])

        # rank[c] = #{c': zb[c'] > zb[c]}  (compare + count fused via accum_out)
        rank = sp.tile([P, NCH], F32, tag="rank")
        for k in range(NCH):
            g = gp.tile([P, C], F32, tag="g")
            nc.vector.tensor_scalar(g[:], zrep_ps[:], zb_sb[:, k : k + 1], None, ALU.is_gt, ALU.add, accum_out=rank[:, k : k + 1])

        # mask: selected iff rank < K
        m = sp.tile([P, NCH], F32, tag="m")
        nc.vector.tensor_scalar(m[:], rank[:], float(K) - 0.5, None, ALU.is_lt)

        # output slot p[c] = #{selected c' < c} (exclusive prefix over channels)
        p_ps = pp.tile([P, NCH], F32, tag="p")
        nc.tensor.matmul(p_ps[:, 0:1], lhsT=sut_sb[:], rhs=m[:, 0:1], start=True, stop=True)
        nc.tensor.matmul(p_ps[:, 1:2], lhsT=ones_sb[:], rhs=m[:, 0:1], start=True, stop=False)
        nc.tensor.matmul(p_ps[:, 1:2], lhsT=sut_sb[:], rhs=m[:, 1:2], start=False, stop=True)

        # q = p + OOB*(1-m); cast to int32 for the scatter offset table
        tmp = sp.tile([P, NCH], F32, tag="tmp")
        nc.vector.tensor_scalar(tmp[:], m[:], -OOB, OOB, ALU.mult, ALU.add)
        qf = sp.tile([P, NCH], F32, tag="qf")
        nc.vector.tensor_tensor(out=qf[:], in0=p_ps[:], in1=tmp[:], op=ALU.add)
        qi = sp.tile([P, NCH], mybir.dt.int32, tag="qi")
        nc.vector.tensor_copy(qi[:], qf[:])

        # scale all channels by attn weight (per-partition scalar on DVE, 2x mode)
        for k in range(NCH):
            nc.vector.tensor_scalar(xs[:, k, :], xt[:, k, :], a_sb[:, k : k + 1], None, ALU.mult)

        # scatter each chunk to out[b] (per-partition row offsets); OOB dropped
        for k in range(NCH):
            nc.gpsimd.indirect_dma_start(
                out=outs_d[b].ap(),
                out_offset=bass.IndirectOffsetOnAxis(ap=qi[:, k : k + 1], axis=0),
                in_=xs[:, k, :],
                in_offset=None,
                bounds_check=K - 1,
                oob_is_err=False,
            )


def build_nc():
    nc = bacc.Bacc("TRN2", target_bir_lowering=False, debug=False, num_devices=N_CORES)
    x_d = nc.dram_tensor("x", [NB, C, HW], F32, kind="ExternalInput")
    w1t_d = nc.dram_tensor("w1t", [C, R], F32, kind="ExternalInput")
    w2t_d = nc.dram_tensor("w2t", [R, C], F32, kind="ExternalInput")
    b1_d = nc.dram_tensor("b1", [R, 1], F32, kind="ExternalInput")
    b2_d = nc.dram_tensor("b2", [NCH, P], F32, kind="ExternalInput")
    sut_d = nc.dram_tensor("sut", [P, P], F32, kind="ExternalInput")
    ones_d = nc.dram_tensor("ones", [P, P], F32, kind="ExternalInput")
    outs_d = [nc.dram_tensor(f"out{b}", [K, HW], F32, kind="ExternalOutput") for b in range(NB)]
    with tile.TileContext(nc) as tc:
        with ExitStack() as ctx:
            _body(ctx, tc, x_d, outs_d, w1t_d, w2t_d, b1_d, b2_d, sut_d, ones_d)
    nc.compile()
    return nc


def make_in_maps(x, w1, b1, w2, b2):
    """Per-core input dicts. x: [32, 256, 64, 64] f32."""
    w1t = np.ascontiguousarray(w1.T).astype(np.float32) / float(HW)  # [C, R], mean folded in
    w2t = np.ascontiguousarray(w2.T).astype(np.float32)  # [R, C]
    b1c = b1.astype(np.float32).reshape(R, 1)
    b2c = b2.astype(np.float32).reshape(NCH, P)
    sut = np.triu(np.ones((P, P), np.float32), k=1)  # sut[k, m] = 1 iff k < m
    ones = np.ones((P, P), np.float32)
    xr = np.ascontiguousarray(x.astype(np.float32).reshape(B_FULL, C, HW))
    in_maps = []
    for i in range(N_CORES):
        in_maps.append(
            {
                "x": np.ascontiguousarray(xr[i * NB : (i + 1) * NB]),
                "w1t": w1t,
                "w2t": w2t,
                "b1": b1c,
                "b2": b2c,
                "sut": sut,
                "ones": ones,
            }
        )
    return in_maps


def _install_ntff_hook():
    """Bridge the missing antenv.axon_hooks module so run_bass_kernel_spmd
    trace=True can capture NTFF profiles via the axon PJRT .so."""
    import sys
    import types

    if "antenv.axon_hooks" in sys.modules:
        return
    try:
        if "/root/.axon_site" not in sys.path:
            sys.path.insert(0, "/root/.axon_site")
        from trn_agent_boot.trn_boot import _ntff_profile_via_ctypes

        hook = _ntff_profile_via_ctypes("/opt/axon/libaxon_pjrt.so")
        mod = types.ModuleType("antenv.axon_hooks")
        mod.get_axon_ntff_profile_hook = lambda: hook
        mod.set_axon_ntff_profile_hook = lambda h: None
        sys.modules["antenv.axon_hooks"] = mod
    except Exception as e:  # degrade to no tracing
        print("ntff hook install failed:", e)


_NC_CACHE = {}


def get_nc():
    if "nc" not in _NC_CACHE:
        _NC_CACHE["nc"] = build_nc()
    return _NC_CACHE["nc"]


def kernel(x, w1, b1, w2, b2, topk, _trace=False, **_ignored):
    assert int(topk) == K, f"kernel hardcodes topk={K}, got {topk}"
    assert x.shape == (B_FULL, C, H, W)
    nc = get_nc()
    if _trace:
        _install_ntff_hook()
    in_maps = make_in_maps(np.asarray(x), np.asarray(w1), np.asarray(b1), np.asarray(w2), np.asarray(b2))
    res = run_bass_kernel_spmd(nc, in_maps, core_ids=list(range(N_CORES)), trace=_trace)
    outs = [np.stack([res.results[i][f"out{b}"] for b in range(NB)]).reshape(NB, K, H, W) for i in range(N_CORES)]
    full = np.concatenate(outs, axis=0).astype(np.float32)
    if _trace:
        return full, res
    return full
